# revision 14
# baseline (speedup 1.0000x reference)
"""Trainium2 Bass kernel for NeuralLongTermMemory (scatter_memory).

Distribution strategy (8 NeuronCores, SPMD):
  - Token-parallel over B*S = 8192 tokens -> 1024 tokens/core. Each core gets
    the same sequence slice of every batch row.
  - All projection / memory-MLP forward+backward GEMMs are token-local.
  - Gate statistics: tiny AllReduce of per-core token-sums of x (16 KB),
    issued at the very start; every core computes alpha/theta/eta redundantly.
  - Memory-MLP gradients: per-core partial g0 (H,D) and g1^T (H,D) are
    ReduceScattered (16 MB -> 2 MB/core); each core applies the
    decay/momentum update to its 1/8 shard of the stacked weights/momentum;
    host concatenates shards.
  - Query/retrieve/out-projection work is emitted after the ReduceScatter
    so it overlaps the collective.

Layouts:
  T-layout [feature_on_partitions, tokens] for forward GEMMs;
  N-layout [tokens_on_partitions, features] for gradient outer products
  (PE transposes convert). Big GEMMs run float32r (full rate, ~tf32).

SBUF is tight (207.87 KB/partition): big activations share four explicit
tag slots (3x32KB + 1x64KB) with strictly sequential tenancy; values / h /
silu'(z0) are spilled to DRAM and re-streamed.
"""

import os
import sys

import numpy as np

sys.path.insert(0, "/opt/trn_rl_repo")

import concourse.bass as bass
from concourse import bacc
import concourse.mybir as mybir
import concourse.tile as tile
from concourse.bass import ts
from concourse.bass_utils import run_bass_kernel_spmd
from concourse.masks import make_identity

B, S, D, H = 4, 2048, 1024, 2048
NCORES = 8
SB = S // NCORES          # 256 sequence positions per batch row per core
T = B * SB                # 1024 local tokens
P = 128
DK, HK, TK = D // P, H // P, T // P   # 8, 16, 8
NT = 512                  # wide moving-dim tile
NW = 256                  # narrow moving-dim tile (weight-streamed stages)
SHARD = 2 * H * D // NCORES           # 524288 elements (2 MB) per core
FS = SHARD // P                       # 4096 free elements per partition
GRAD_SCALE = 2.0 / (B * S * D)

FP = mybir.dt.float32
FPR = mybir.dt.float32r
AF = mybir.ActivationFunctionType
ALU = mybir.AluOpType

CORE_IDS = list(range(NCORES))


def r(ap):
    """fp32 -> float32r view for full-rate PE matmuls."""
    return ap.bitcast(FPR)


def _build_program():
    from contextlib import ExitStack

    nc = bacc.Bacc(None, num_devices=NCORES)

    # ---------------- I/O declarations ----------------
    xT_h = nc.dram_tensor("xT", [D, T], FP, kind="ExternalInput")
    w_h = {}
    for name in ["kp1T", "kp2T", "vp1T", "vp2T", "qp1T", "qp2T",
                 "gdT", "glT", "gmT", "owT"]:
        w_h[name] = nc.dram_tensor(name, [D, D], FP, kind="ExternalInput")
    b_h = {}
    for name in ["gdb", "glb", "gmb"]:
        b_h[name] = nc.dram_tensor(name, [D], FP, kind="ExternalInput")
    mw0T_h = nc.dram_tensor("mw0T", [D, H], FP, kind="ExternalInput")
    mw1T_h = nc.dram_tensor("mw1T", [H, D], FP, kind="ExternalInput")
    mw1n_h = nc.dram_tensor("mw1n", [D, H], FP, kind="ExternalInput")
    wsh_h = nc.dram_tensor("wsh", [SHARD], FP, kind="ExternalInput")
    msh_h = nc.dram_tensor("msh", [SHARD], FP, kind="ExternalInput")

    outN_h = nc.dram_tensor("outN", [T, D], FP, kind="ExternalOutput")
    nwsh_h = nc.dram_tensor("nwsh", [SHARD], FP, kind="ExternalOutput")
    nmsh_h = nc.dram_tensor("nmsh", [SHARD], FP, kind="ExternalOutput")
    dbg_g_h = None
    if os.environ.get("K_DEBUG_G"):
        dbg_g_h = nc.dram_tensor("dbg_g", [SHARD], FP, kind="ExternalOutput")

    def w4ap(h):        # (D, N) -> [p, dk, n]
        return h[:, :].rearrange("(dk p) n -> p dk n", p=P)

    mw0T_ap = mw0T_h[:, :].rearrange("(dk p) h -> p dk h", p=P)
    mw1T_ap = mw1T_h[:, :].rearrange("(hk p) d -> p hk d", p=P)
    mw1n_ap = mw1n_h[:, :].rearrange("(dk p) h -> p dk h", p=P)

    with tile.TileContext(nc, num_cores=NCORES) as tc, ExitStack() as stack:
        sb = stack.enter_context(tc.tile_pool(name="sb", bufs=1))
        ps = stack.enter_context(tc.tile_pool(name="ps", bufs=1, space="PSUM"))
        dr = stack.enter_context(tc.tile_pool(name="dr", bufs=1, space="DRAM"))

        # ---------------- constants ----------------
        ident = sb.tile([P, P], FP, tag="ident")
        make_identity(nc, ident[:])
        ones_col_f = sb.tile([P, 1], FP, tag="ones_col_f")
        nc.vector.memset(ones_col_f[:], 1.0)
        ones_col = sb.tile([P, 1], FPR, tag="ones_col")
        nc.vector.tensor_copy(ones_col[:], ones_col_f[:])
        ones_row = sb.tile([1, P], FP, tag="ones_row")
        nc.vector.memset(ones_row[:], 1.0)

        # ---------------- stage 1: load x, token-sums, AllReduce ----------
        X = sb.tile([P, DK, T], FPR, tag="slotA")
        nc.sync.dma_start(
            X[:], xT_h[:, :].rearrange("(dk p) t -> p dk t", p=P).bitcast(FPR)
        )

        xs_sb = sb.tile([P, DK, B], FP, tag="xsum")
        nc.vector.reduce_sum(
            xs_sb[:],
            X[:].bitcast(FP).rearrange("p dk (b s) -> p dk b s", s=SB),
            axis=mybir.AxisListType.X,
        )
        xsum_in = dr.tile([D, B], FP)
        xsum_out = dr.tile([D, B], FP, addr_space="Shared")
        nc.sync.dma_start(
            xsum_in[:].rearrange("(dk p) b -> p dk b", p=P), xs_sb[:]
        )
        nc.gpsimd.collective_compute(
            "AllReduce", ALU.add, replica_groups=[CORE_IDS],
            ins=[xsum_in.opt()], outs=[xsum_out.opt()],
        )

        # ---------------- helpers ----------------
        def norm_inplace(dest):
            """dest <- dest / max(||dest||_2(feature axis), 1e-12), per token."""
            invBs = []
            for n in range(T // NT):
                pn = ps.tile([1, NT], FP, tag="nrm", bufs=1)
                for k in range(DK):
                    sq = sb.tile([P, NT], FPR, tag="gst", bufs=2)
                    nc.vector.tensor_mul(
                        sq[:], dest[:, k, ts(n, NT)], dest[:, k, ts(n, NT)]
                    )
                    nc.tensor.matmul(
                        pn[:], ones_col[:], sq[:],
                        start=(k == 0), stop=(k == DK - 1),
                    )
                inv = sb.tile([1, NT], FP, tag="gst", bufs=2)
                nc.scalar.sqrt(inv[:], pn[:])
                nc.vector.tensor_scalar_max(inv[:], inv[:], 1e-12)
                nc.vector.reciprocal(inv[:], inv[:])
                pb = ps.tile([P, NT], FP, tag="mm", bufs=4)
                nc.tensor.matmul(pb[:], ones_row[:], inv[:],
                                 start=True, stop=True)
                invB = sb.tile([P, NT], FPR, tag="hn_ring", bufs=2)
                nc.vector.tensor_copy(invB[:], pb[:])
                invBs.append(invB)
            for k in range(DK):
                for n in range(T // NT):
                    nc.vector.tensor_mul(
                        dest[:, k, ts(n, NT)], dest[:, k, ts(n, NT)], invBs[n][:]
                    )

        def projection(xin, w1name, w2name, dest, spill_to=None):
            """dest <- silu(silu(x @ w1.T) @ w2.T), T-layout.
            If spill_to (DRAM [DK, P, T]) is given, result slices are DMA'd
            there via a ring instead of requiring dest (pass dest=None)."""
            a1 = sb.tile([P, DK, T], FPR, tag="slotB")
            for wname, rhs, dst in ((w1name, xin, a1), (w2name, a1, dest)):
                for m in range(DK):
                    Wc = sb.tile([P, DK, P], FPR, tag="w4c", bufs=2)
                    nc.sync.dma_start(
                        Wc[:], w4ap(w_h[wname])[:, :, ts(m, P)].bitcast(FPR)
                    )
                    for n in range(T // NT):
                        pt = ps.tile([P, NT], FP, tag="mm", bufs=4)
                        for k in range(DK):
                            nc.tensor.matmul(
                                pt[:], Wc[:, k, :], rhs[:, k, ts(n, NT)],
                                start=(k == 0), stop=(k == DK - 1),
                            )
                        if dst is not None:
                            nc.scalar.activation(
                                dst[:, m, ts(n, NT)], pt[:], AF.Silu
                            )
                        else:
                            vt = sb.tile([P, NT], FP, tag="sp_ring", bufs=2)
                            nc.scalar.activation(vt[:], pt[:], AF.Silu)
                            nc.sync.dma_start(spill_to[m, :, ts(n, NT)], vt[:])

        # ---------------- stage 2: keys (resident+norm), values (spilled) -
        keysT = sb.tile([P, DK, T], FPR, tag="keysT")
        projection(X, "kp1T", "kp2T", keysT)
        norm_inplace(keysT)
        val_spill = dr.tile([DK, P, T], FP)
        projection(X, "vp1T", "vp2T", None, spill_to=val_spill)

        # ---------------- stage 3: z0 = keys @ w0^T; h=silu, sp=silu' -----
        # hT kept in SBUF; hN and sp spilled to DRAM.
        hT = sb.tile([P, HK, T], FPR, tag="slotE")
        hN_spill = dr.tile([TK, P, H], FP)
        sp_spill = dr.tile([TK, P, H], FP)
        for hn in range(H // NW):
            MW0 = sb.tile([P, DK, NW], FPR, tag="w2m", bufs=2)
            nc.sync.dma_start(MW0[:], mw0T_ap[:, :, ts(hn, NW)].bitcast(FPR))
            for m in range(TK):
                pt = ps.tile([P, NW], FP, tag="mm", bufs=4)
                for k in range(DK):
                    nc.tensor.matmul(
                        pt[:], keysT[:, k, ts(m, P)], MW0[:, k, :],
                        start=(k == 0), stop=(k == DK - 1),
                    )
                hn_t = sb.tile([P, NW], FP, tag="hn_ring", bufs=2)
                nc.scalar.activation(hn_t[:], pt[:], AF.Silu)
                sp_t = sb.tile([P, NW], FP, tag="sp_ring", bufs=2)
                nc.scalar.activation(sp_t[:], pt[:], AF.Derivative_silu)
                nc.sync.dma_start(hN_spill[m, :, ts(hn, NW)], hn_t[:])
                nc.sync.dma_start(sp_spill[m, :, ts(hn, NW)], sp_t[:])
                for j in range(NW // P):
                    tp = ps.tile([P, P], FP, tag="tr", bufs=2)
                    nc.tensor.transpose(tp[:], hn_t[:, ts(j, P)], ident[:])
                    nc.vector.tensor_copy(
                        hT[:, hn * (NW // P) + j, ts(m, P)], tp[:]
                    )

        # ---------------- stage 4: pred^T; dpred^T = pred^T - values^T ----
        dpredT = sb.tile([P, DK, T], FPR, tag="slotA")
        for m in range(DK):
            pts = [ps.tile([P, NT], FP, tag="mm", bufs=4, name=f"pred_ps{_n}") for _n in range(T // NT)]
            for kh in range(2):
                W1C = sb.tile([P, HK // 2, P], FPR, tag="w1m", bufs=2)
                nc.sync.dma_start(
                    W1C[:], mw1T_ap[:, ts(kh, HK // 2), ts(m, P)].bitcast(FPR)
                )
                for n in range(T // NT):
                    for k8 in range(HK // 2):
                        nc.tensor.matmul(
                            pts[n][:], W1C[:, k8, :],
                            hT[:, kh * (HK // 2) + k8, ts(n, NT)],
                            start=(kh == 0 and k8 == 0),
                            stop=(kh == 1 and k8 == HK // 2 - 1),
                        )
            for n in range(T // NT):
                vt = sb.tile([P, NT], FP, tag="sp_ring", bufs=2)
                nc.sync.dma_start(vt[:], val_spill[m, :, ts(n, NT)])
                nc.vector.tensor_tensor(
                    dpredT[:, m, ts(n, NT)], pts[n][:], vt[:], ALU.subtract
                )

        # ---------------- stage 5: dpredN = transpose(dpredT) -------------
        dpredN = sb.tile([P, TK, D], FPR, tag="slotB")
        for dk in range(DK):
            for tk in range(TK):
                tp = ps.tile([P, P], FP, tag="tr", bufs=2)
                nc.tensor.transpose(
                    tp[:], dpredT[:, dk, ts(tk, P)].bitcast(FP), ident[:]
                )
                nc.vector.tensor_copy(dpredN[:, tk, ts(dk, P)], tp[:])

        # ---------------- stage 6: reload hN; g1^T partials ---------------
        hN = sb.tile([P, TK, H], FPR, tag="slotE")
        nc.sync.dma_start(
            hN[:], hN_spill[:].rearrange("tk p h -> p tk h").bitcast(FPR)
        )
        g_in = dr.tile([2, H, D], FP)
        for m in range(HK):
            for n in range(D // NT):
                pt = ps.tile([P, NT], FP, tag="mm", bufs=4)
                for k in range(TK):
                    nc.tensor.matmul(
                        pt[:], hN[:, k, ts(m, P)], dpredN[:, k, ts(n, NT)],
                        start=(k == 0), stop=(k == TK - 1),
                    )
                st = sb.tile([P, NT], FP, tag="gst", bufs=2)
                nc.vector.tensor_copy(st[:], pt[:])
                nc.sync.dma_start(g_in[1, ts(m, P), ts(n, NT)], st[:])

        # ---------------- stage 7: dh (N-layout); dpre = dh * sp ----------
        dpreN = sb.tile([P, TK, H], FPR, tag="slotE")
        for hn in range(H // NW):
            MW1N = sb.tile([P, DK, NW], FPR, tag="w2m", bufs=2)
            nc.sync.dma_start(MW1N[:], mw1n_ap[:, :, ts(hn, NW)].bitcast(FPR))
            for m in range(TK):
                pt = ps.tile([P, NW], FP, tag="mm", bufs=4)
                for k in range(DK):
                    nc.tensor.matmul(
                        pt[:], dpredT[:, k, ts(m, P)], MW1N[:, k, :],
                        start=(k == 0), stop=(k == DK - 1),
                    )
                sp_t = sb.tile([P, NW], FP, tag="sp_ring", bufs=2)
                nc.sync.dma_start(sp_t[:], sp_spill[m, :, ts(hn, NW)])
                nc.vector.tensor_tensor(
                    dpreN[:, m, ts(hn, NW)], pt[:], sp_t[:], ALU.mult
                )

        # ---------------- stage 8: keysN; g0 partials ---------------------
        keysN = sb.tile([P, TK, D], FPR, tag="slotA")
        for dk in range(DK):
            for tk in range(TK):
                tp = ps.tile([P, P], FP, tag="tr", bufs=2)
                nc.tensor.transpose(
                    tp[:], keysT[:, dk, ts(tk, P)].bitcast(FP), ident[:]
                )
                nc.vector.tensor_copy(keysN[:, tk, ts(dk, P)], tp[:])
        for m in range(HK):
            for n in range(D // NT):
                pt = ps.tile([P, NT], FP, tag="mm", bufs=4)
                for k in range(TK):
                    nc.tensor.matmul(
                        pt[:], dpreN[:, k, ts(m, P)], keysN[:, k, ts(n, NT)],
                        start=(k == 0), stop=(k == TK - 1),
                    )
                st = sb.tile([P, NT], FP, tag="gst", bufs=2)
                nc.vector.tensor_copy(st[:], pt[:])
                nc.sync.dma_start(g_in[0, ts(m, P), ts(n, NT)], st[:])

        # ---------------- stage 9: ReduceScatter gradients ----------------
        g_out = dr.tile([SHARD], FP)
        nc.gpsimd.collective_compute(
            "ReduceScatter", ALU.add, replica_groups=[CORE_IDS],
            ins=[g_in.opt()], outs=[g_out.opt()],
        )
        if dbg_g_h is not None:
            nc.sync.dma_start(dbg_g_h[:], g_out[:])

        # ---------------- stage 10: queries / retrieved / out -------------
        X2 = sb.tile([P, DK, T], FPR, tag="slotA")
        nc.sync.dma_start(
            X2[:], xT_h[:, :].rearrange("(dk p) t -> p dk t", p=P).bitcast(FPR)
        )
        queriesT = sb.tile([P, DK, T], FPR, tag="keysT")
        projection(X2, "qp1T", "qp2T", queriesT)
        norm_inplace(queriesT)

        qhT = sb.tile([P, HK, T], FPR, tag="slotE")
        for m in range(HK):
            W0C = sb.tile([P, DK, P], FPR, tag="w4c", bufs=2)
            nc.sync.dma_start(W0C[:], mw0T_ap[:, :, ts(m, P)].bitcast(FPR))
            for n in range(T // NT):
                pt = ps.tile([P, NT], FP, tag="mm", bufs=4)
                for k in range(DK):
                    nc.tensor.matmul(
                        pt[:], W0C[:, k, :], queriesT[:, k, ts(n, NT)],
                        start=(k == 0), stop=(k == DK - 1),
                    )
                nc.scalar.activation(qhT[:, m, ts(n, NT)], pt[:], AF.Silu)

        retrT = sb.tile([P, DK, T], FPR, tag="slotA")
        for m in range(DK):
            pts = [ps.tile([P, NT], FP, tag="mm", bufs=4, name=f"pred_ps{_n}") for _n in range(T // NT)]
            for kh in range(2):
                W1C = sb.tile([P, HK // 2, P], FPR, tag="w1m", bufs=2)
                nc.sync.dma_start(
                    W1C[:], mw1T_ap[:, ts(kh, HK // 2), ts(m, P)].bitcast(FPR)
                )
                for n in range(T // NT):
                    for k8 in range(HK // 2):
                        nc.tensor.matmul(
                            pts[n][:], W1C[:, k8, :],
                            qhT[:, kh * (HK // 2) + k8, ts(n, NT)],
                            start=(kh == 0 and k8 == 0),
                            stop=(kh == 1 and k8 == HK // 2 - 1),
                        )
            for n in range(T // NT):
                nc.vector.tensor_copy(retrT[:, m, ts(n, NT)], pts[n][:])

        outN_ap = outN_h[:, :].rearrange("(tk p) d -> p tk d", p=P)
        for nd in range(D // NW):
            OWT = sb.tile([P, DK, NW], FPR, tag="w2m", bufs=2)
            nc.sync.dma_start(
                OWT[:], w4ap(w_h["owT"])[:, :, ts(nd, NW)].bitcast(FPR)
            )
            for m in range(TK):
                pt = ps.tile([P, NW], FP, tag="mm", bufs=4)
                for k in range(DK):
                    nc.tensor.matmul(
                        pt[:], retrT[:, k, ts(m, P)], OWT[:, k, :],
                        start=(k == 0), stop=(k == DK - 1),
                    )
                st = sb.tile([P, NW], FP, tag="gst", bufs=2)
                nc.vector.tensor_copy(st[:], pt[:])
                nc.sync.dma_start(outN_ap[:, m, ts(nd, NW)], st[:])

        # ---------------- stage 11: gates ---------------------------------
        xsum_sb = sb.tile([P, DK, B], FP, tag="xsum")
        nc.sync.dma_start(
            xsum_sb[:], xsum_out[:].rearrange("(dk p) b -> p dk b", p=P)
        )
        red = sb.tile([P, 3], FP, tag="red")
        for gi, (wname, bname) in enumerate(
            (("gdT", "gdb"), ("glT", "glb"), ("gmT", "gmb"))
        ):
            gb = sb.tile([P, DK], FP, tag="gb", bufs=2)
            nc.sync.dma_start(
                gb[:], b_h[bname][:].rearrange("(dk p) -> p dk", p=P)
            )
            sg = sb.tile([P, DK, B], FP, tag="sg", bufs=2)
            for m in range(DK):
                GWc = sb.tile([P, DK, P], FP, tag="w4c", bufs=2)
                nc.sync.dma_start(GWc[:], w4ap(w_h[wname])[:, :, ts(m, P)])
                pg = ps.tile([P, B], FP, tag="psg", bufs=1)
                for k in range(DK):
                    nc.tensor.matmul(
                        pg[:], GWc[:, k, :], xsum_sb[:, k, :],
                        start=(k == 0), stop=(k == DK - 1),
                    )
                nc.scalar.activation(
                    sg[:, m, :], pg[:], AF.Sigmoid,
                    bias=gb[:, m : m + 1], scale=1.0 / S,
                )
            nc.vector.reduce_sum(
                red[:, gi : gi + 1], sg[:], axis=mybir.AxisListType.XY
            )
        p3 = ps.tile([1, 3], FP, tag="nrm", bufs=1)
        nc.tensor.matmul(
            p3[:], ones_col[:].bitcast(FP), red[:], start=True, stop=True
        )
        srow = sb.tile([1, 3], FP, tag="srow")
        nc.scalar.mul(srow[:], p3[:], 1.0 / (B * D))
        pb3 = ps.tile([P, 3], FP, tag="nrm", bufs=1)
        nc.tensor.matmul(pb3[:], ones_row[:], srow[:], start=True, stop=True)
        coef = sb.tile([P, 3], FP, tag="coef")
        # coef[:,0]=1-alpha ; coef[:,1]=theta*GRAD_SCALE ; coef[:,2]=eta
        nc.vector.tensor_scalar(
            coef[:, 0:1], pb3[:, 0:1], -1.0, 1.0, ALU.mult, ALU.add
        )
        nc.vector.tensor_scalar_mul(coef[:, 1:2], pb3[:, 1:2], GRAD_SCALE)
        nc.vector.tensor_copy(coef[:, 2:3], pb3[:, 2:3])

        # ---------------- stage 12: momentum / weight update --------------
        upd_a = sb.tile([P, 2, FS], FP, tag="slotB")   # [g, mom]
        nc.sync.dma_start(upd_a[:, 0, :], g_out[:].rearrange("(p f) -> p f", p=P))
        nc.sync.dma_start(upd_a[:, 1, :], msh_h[:].rearrange("(p f) -> p f", p=P))
        upd_b = sb.tile([P, 2, FS], FP, tag="keysT")   # [w, new_m]
        nc.sync.dma_start(upd_b[:, 0, :], wsh_h[:].rearrange("(p f) -> p f", p=P))

        nc.vector.tensor_scalar_mul(upd_a[:, 0, :], upd_a[:, 0, :], coef[:, 1:2])
        nc.vector.scalar_tensor_tensor(
            upd_b[:, 1, :], upd_a[:, 1, :], coef[:, 2:3], upd_a[:, 0, :],
            ALU.mult, ALU.subtract,
        )
        nc.vector.scalar_tensor_tensor(
            upd_b[:, 0, :], upd_b[:, 0, :], coef[:, 0:1], upd_b[:, 1, :],
            ALU.mult, ALU.add,
        )
        nc.sync.dma_start(
            nmsh_h[:].rearrange("(p f) -> p f", p=P), upd_b[:, 1, :]
        )
        nc.sync.dma_start(
            nwsh_h[:].rearrange("(p f) -> p f", p=P), upd_b[:, 0, :]
        )

    nc.finalize()
    return nc


_NC_CACHE = {}


def _get_nc():
    if "nc" not in _NC_CACHE:
        _NC_CACHE["nc"] = _build_program()
    return _NC_CACHE["nc"]


def _prep_in_maps(inputs):
    f32c = lambda a: np.ascontiguousarray(np.asarray(a, dtype=np.float32))
    x = f32c(inputs["x"])
    shared = {
        "kp1T": f32c(inputs["kp_w1"].T), "kp2T": f32c(inputs["kp_w2"].T),
        "vp1T": f32c(inputs["vp_w1"].T), "vp2T": f32c(inputs["vp_w2"].T),
        "qp1T": f32c(inputs["qp_w1"].T), "qp2T": f32c(inputs["qp_w2"].T),
        "gdT": f32c(inputs["gd_w"].T), "glT": f32c(inputs["gl_w"].T),
        "gmT": f32c(inputs["gm_w"].T),
        "gdb": f32c(inputs["gd_b"]), "glb": f32c(inputs["gl_b"]),
        "gmb": f32c(inputs["gm_b"]),
        "mw0T": f32c(inputs["mem_w0"].T),   # (D, H)
        "mw1T": f32c(inputs["mem_w1"].T),   # (H, D)
        "mw1n": f32c(inputs["mem_w1"]),     # (D, H)
        "owT": f32c(inputs["out_w"].T),
    }
    wstack = np.concatenate(
        [f32c(inputs["mem_w0"]).reshape(H, D),
         f32c(inputs["mem_w1"]).T.reshape(H, D)]
    ).reshape(-1)
    mstack = np.concatenate(
        [f32c(inputs["mom0"]).reshape(H, D),
         f32c(inputs["mom1"]).T.reshape(H, D)]
    ).reshape(-1)

    in_maps = []
    for c in range(NCORES):
        xs = x[:, c * SB : (c + 1) * SB, :].reshape(T, D)
        m = dict(shared)
        m["xT"] = np.ascontiguousarray(xs.T)
        m["wsh"] = np.ascontiguousarray(wstack[c * SHARD : (c + 1) * SHARD])
        m["msh"] = np.ascontiguousarray(mstack[c * SHARD : (c + 1) * SHARD])
        in_maps.append(m)
    return in_maps


def _unshard(results):
    out = np.empty((B, S, D), dtype=np.float32)
    for c in range(NCORES):
        o = results[c]["outN"].reshape(B, SB, D)
        out[:, c * SB : (c + 1) * SB, :] = o
    nw = np.concatenate([results[c]["nwsh"] for c in range(NCORES)]).reshape(2 * H, D)
    nm = np.concatenate([results[c]["nmsh"] for c in range(NCORES)]).reshape(2 * H, D)
    new_w0 = nw[:H]                                  # (H, D)
    new_w1 = np.ascontiguousarray(nw[H:].T)          # (D, H)
    new_m0 = nm[:H]
    new_m1 = np.ascontiguousarray(nm[H:].T)
    return out, new_w0, new_w1, new_m0, new_m1


def kernel(**inputs):
    nc = _get_nc()
    in_maps = _prep_in_maps(inputs)
    res = run_bass_kernel_spmd(nc, in_maps, CORE_IDS)
    return _unshard(res.results)


def run_traced(**inputs):
    """Like kernel() but with NTFF tracing; returns (outputs, BassKernelResults)."""
    nc = _get_nc()
    in_maps = _prep_in_maps(inputs)
    res = run_bass_kernel_spmd(nc, in_maps, CORE_IDS, trace=True)
    return _unshard(res.results), res


# revision 16
# speedup vs baseline: 1.0537x; 1.0537x over previous
"""Trainium2 Bass kernel for NeuralLongTermMemory (scatter_memory).

Distribution (8 NeuronCores, SPMD):
  - Token-parallel over B*S = 8192 tokens -> 1024/core (same sequence slice of
    every batch row per core).
  - Gate statistics: tiny AllReduce of per-core token-sums of x, issued first;
    every core computes alpha/theta/eta redundantly.
  - Memory-MLP gradients g0 (H,D) and g1^T (H,D): local partials are
    ReduceScattered (16 MB -> 2 MB/core); each core updates its 1/8 shard of
    the stacked weights/momentum; host concatenates shards.
  - The query/retrieve/out-projection compute is scheduled to overlap the
    ReduceScatter.

Layouts: T-layout [feature, tokens] for forward GEMMs, N-layout
[tokens, feature] for gradient outer products (PE transposes convert).
All big GEMMs run float32r (full PE rate, ~tf32 accuracy), fp32 PSUM.
Weights are host-prearranged into the exact SBUF tile layouts so every DMA is
per-partition contiguous.
"""

import os
import sys

import numpy as np

sys.path.insert(0, "/opt/trn_rl_repo")

import concourse.bass as bass
from concourse import bacc
import concourse.mybir as mybir
import concourse.tile as tile
from concourse.bass import ts
from concourse.bass_utils import run_bass_kernel_spmd
from concourse.masks import make_identity

B, S, D, H = 4, 2048, 1024, 2048
NCORES = 8
SB = S // NCORES          # 256 sequence positions per batch row per core
T = B * SB                # 1024 local tokens
P = 128
DK, HK, TK = D // P, H // P, T // P   # 8, 16, 8
NT = 512                  # wide moving-dim tile
NW = 256                  # narrow moving-dim tile (weight-streamed stages)
SHARD = 2 * H * D // NCORES           # 524288 elements (2 MB) per core
FS = SHARD // P                       # 4096 free elements per partition
GRAD_SCALE = 2.0 / (B * S * D)

FP = mybir.dt.float32
FPR = mybir.dt.float32r
AF = mybir.ActivationFunctionType
ALU = mybir.AluOpType

CORE_IDS = list(range(NCORES))


def _build_program():
    from contextlib import ExitStack

    nc = bacc.Bacc(None, num_devices=NCORES)

    # ---------------- I/O declarations ----------------
    # All prearranged on host so each DMA slice is per-partition contiguous.
    xT_h = nc.dram_tensor("xTp", [P, DK, T], FP, kind="ExternalInput")
    wp_h = {}
    for name in ["kp1", "kp2", "vp1", "vp2", "qp1", "qp2"]:
        wp_h[name] = nc.dram_tensor(name + "p", [DK, P, DK, P], FP,
                                    kind="ExternalInput")
    gw_h = {}
    for name in ["gd", "gl", "gm"]:
        gw_h[name] = nc.dram_tensor(name + "p", [D // NW, P, DK, NW], FP,
                                    kind="ExternalInput")
    gb_h = {}
    for name in ["gdb", "glb", "gmb"]:
        gb_h[name] = nc.dram_tensor(name, [D], FP, kind="ExternalInput")
    mw0z_h = nc.dram_tensor("mw0zp", [H // NW, P, DK, NW], FP,
                            kind="ExternalInput")
    mw0q_h = nc.dram_tensor("mw0qp", [HK, P, DK, P], FP, kind="ExternalInput")
    mw1T_h = nc.dram_tensor("mw1Tp", [DK, 2, P, HK // 2, P], FP,
                            kind="ExternalInput")
    mw1n_h = nc.dram_tensor("mw1np", [H // NW, P, DK, NW], FP,
                            kind="ExternalInput")
    ow_h = nc.dram_tensor("owp", [D // NW, P, DK, NW], FP, kind="ExternalInput")
    wsh_h = nc.dram_tensor("wsh", [SHARD], FP, kind="ExternalInput")
    msh_h = nc.dram_tensor("msh", [SHARD], FP, kind="ExternalInput")

    outN_h = nc.dram_tensor("outN", [T, D], FP, kind="ExternalOutput")
    nwsh_h = nc.dram_tensor("nwsh", [SHARD], FP, kind="ExternalOutput")
    nmsh_h = nc.dram_tensor("nmsh", [SHARD], FP, kind="ExternalOutput")
    dbg_g_h = None
    if os.environ.get("K_DEBUG_G"):
        dbg_g_h = nc.dram_tensor("dbg_g", [SHARD], FP, kind="ExternalOutput")

    with tile.TileContext(nc, num_cores=NCORES) as tc, ExitStack() as stack:
        sb = stack.enter_context(tc.tile_pool(name="sb", bufs=1))
        ps = stack.enter_context(tc.tile_pool(name="ps", bufs=1, space="PSUM"))
        dr = stack.enter_context(tc.tile_pool(name="dr", bufs=1, space="DRAM"))

        # ---------------- constants ----------------
        ident = sb.tile([P, P], FP, tag="ident")
        make_identity(nc, ident[:])
        ones_col_f = sb.tile([P, 1], FP, tag="ones_col_f")
        nc.vector.memset(ones_col_f[:], 1.0)
        ones_col = sb.tile([P, 1], FPR, tag="ones_col")
        nc.vector.tensor_copy(ones_col[:], ones_col_f[:])
        ones_row = sb.tile([1, P], FP, tag="ones_row")
        nc.vector.memset(ones_row[:], 1.0)
        ones1x4 = sb.tile([1, B], FPR, tag="ones1x4")
        nc.vector.tensor_copy(ones1x4[:], ones_col_f[0:1, 0:1].to_broadcast([1, B]))

        # ---------------- stage 1: load x, token-sums, AllReduce ----------
        X = sb.tile([P, DK, T], FPR, tag="slotA")
        nc.sync.dma_start(X[:], xT_h[:].bitcast(FPR))

        xs_sb = sb.tile([P, DK, B], FP, tag="xsum")
        nc.vector.reduce_sum(
            xs_sb[:],
            X[:].bitcast(FP).rearrange("p dk (b s) -> p dk b s", s=SB),
            axis=mybir.AxisListType.X,
        )
        xsum_in = dr.tile([D, B], FP)
        xsum_out = dr.tile([D, B], FP, addr_space="Shared")
        nc.sync.dma_start(
            xsum_in[:].rearrange("(dk p) b -> p dk b", p=P), xs_sb[:]
        )
        nc.gpsimd.collective_compute(
            "AllReduce", ALU.add, replica_groups=[CORE_IDS],
            ins=[xsum_in.opt()], outs=[xsum_out.opt()],
        )

        # ---------------- helpers ----------------
        def norm_inplace(dest):
            """dest <- dest / max(||dest||_2(feature axis), 1e-12), per token."""
            invBs = []
            for n in range(T // NT):
                pn = ps.tile([1, NT], FP, tag="nrm", bufs=1)
                for k in range(DK):
                    sq = sb.tile([P, NT], FPR, tag="gst", bufs=2)
                    nc.vector.tensor_mul(
                        sq[:], dest[:, k, ts(n, NT)], dest[:, k, ts(n, NT)]
                    )
                    nc.tensor.matmul(
                        pn[:], ones_col[:], sq[:],
                        start=(k == 0), stop=(k == DK - 1),
                    )
                inv = sb.tile([1, NT], FP, tag="gst", bufs=2)
                nc.scalar.sqrt(inv[:], pn[:])
                nc.vector.tensor_scalar_max(inv[:], inv[:], 1e-12)
                nc.vector.reciprocal(inv[:], inv[:])
                pb = ps.tile([P, NT], FP, tag="mm", bufs=4)
                nc.tensor.matmul(pb[:], ones_row[:], inv[:],
                                 start=True, stop=True)
                invB = sb.tile([P, NT], FPR, tag="hn_ring", bufs=2)
                nc.vector.tensor_copy(invB[:], pb[:])
                invBs.append(invB)
            for k in range(DK):
                for n in range(T // NT):
                    nc.vector.tensor_mul(
                        dest[:, k, ts(n, NT)], dest[:, k, ts(n, NT)], invBs[n][:]
                    )

        def projection(xin, w1name, w2name, dest, a1_tag, spill_to=None):
            """dest <- silu(silu(x @ w1.T) @ w2.T), T-layout.
            If spill_to (DRAM [DK, P, T]) given, slices DMA there instead."""
            a1 = sb.tile([P, DK, T], FPR, tag=a1_tag)
            for wname, rhs, dst in ((w1name, xin, a1), (w2name, a1, dest)):
                for m in range(DK):
                    Wc = sb.tile([P, DK, P], FPR, tag="w4c", bufs=2)
                    nc.sync.dma_start(Wc[:], wp_h[wname][m].bitcast(FPR))
                    for n in range(T // NT):
                        pt = ps.tile([P, NT], FP, tag="mm", bufs=4)
                        for k in range(DK):
                            nc.tensor.matmul(
                                pt[:], Wc[:, k, :], rhs[:, k, ts(n, NT)],
                                start=(k == 0), stop=(k == DK - 1),
                            )
                        if dst is not None:
                            nc.scalar.activation(
                                dst[:, m, ts(n, NT)], pt[:], AF.Silu
                            )
                        else:
                            vt = sb.tile([P, NT], FP, tag="sp_ring", bufs=2)
                            nc.scalar.activation(vt[:], pt[:], AF.Silu)
                            nc.sync.dma_start(spill_to[m, :, ts(n, NT)], vt[:])

        # ---------------- stage 2: keys (resident+norm), values (spilled) -
        keysT = sb.tile([P, DK, T], FPR, tag="keysT")
        projection(X, "kp1", "kp2", keysT, "slotB")
        norm_inplace(keysT)
        val_spill = dr.tile([DK, P, T], FP)
        projection(X, "vp1", "vp2", None, "slotB", spill_to=val_spill)

        # ---------------- stage 3: z0 = keys @ w0^T; h=silu, sp=silu' -----
        # hT kept in SBUF; hN and sp spilled to DRAM.
        # silu/silu' via one Sigmoid ACT pass + DVE algebra (no LUT swaps):
        #   sg = sigmoid(z); h = z*sg; sp = sg*(1 + z - h)
        hT = sb.tile([P, HK, T], FPR, tag="slotE")
        hN_spill = dr.tile([TK, P, H], FP)
        sp_spill = dr.tile([TK, P, H], FP)
        for hn in range(H // NW):
            MW0 = sb.tile([P, DK, NW], FPR, tag="w2m", bufs=2)
            nc.sync.dma_start(MW0[:], mw0z_h[hn].bitcast(FPR))
            for m in range(TK):
                pt = ps.tile([P, NW], FP, tag="mm", bufs=4)
                for k in range(DK):
                    nc.tensor.matmul(
                        pt[:], keysT[:, k, ts(m, P)], MW0[:, k, :],
                        start=(k == 0), stop=(k == DK - 1),
                    )
                sg_t = sb.tile([P, NW], FP, tag="sg_ring", bufs=2)
                nc.scalar.activation(sg_t[:], pt[:], AF.Sigmoid)
                hn_t = sb.tile([P, NW], FP, tag="hn_ring", bufs=2)
                nc.vector.tensor_tensor(hn_t[:], pt[:], sg_t[:], ALU.mult)
                nc.sync.dma_start(hN_spill[m, :, ts(hn, NW)], hn_t[:])
                sp_t = sb.tile([P, NW], FP, tag="sp_ring", bufs=2)
                nc.vector.scalar_tensor_tensor(
                    sp_t[:], pt[:], 1.0, hn_t[:], ALU.add, ALU.subtract
                )
                nc.vector.tensor_tensor(sp_t[:], sp_t[:], sg_t[:], ALU.mult)
                nc.sync.dma_start(sp_spill[m, :, ts(hn, NW)], sp_t[:])
                for j in range(NW // P):
                    tp = ps.tile([P, P], FP, tag="tr", bufs=3)
                    nc.tensor.transpose(tp[:], hn_t[:, ts(j, P)], ident[:])
                    nc.vector.tensor_copy(
                        hT[:, hn * (NW // P) + j, ts(m, P)], tp[:]
                    )

        # ---------------- stage 4: pred^T; dpred^T = pred^T - values^T ----
        dpredT = sb.tile([P, DK, T], FPR, tag="slotA")
        for m in range(DK):
            pts = [ps.tile([P, NT], FP, tag="mm", bufs=4, name=f"pred_ps{_n}")
                   for _n in range(T // NT)]
            for kh in range(2):
                W1C = sb.tile([P, HK // 2, P], FPR, tag="w1m", bufs=2)
                nc.sync.dma_start(W1C[:], mw1T_h[m, kh].bitcast(FPR))
                for n in range(T // NT):
                    for k8 in range(HK // 2):
                        nc.tensor.matmul(
                            pts[n][:], W1C[:, k8, :],
                            hT[:, kh * (HK // 2) + k8, ts(n, NT)],
                            start=(kh == 0 and k8 == 0),
                            stop=(kh == 1 and k8 == HK // 2 - 1),
                        )
            for n in range(T // NT):
                vt = sb.tile([P, NT], FP, tag="sp_ring", bufs=2)
                nc.sync.dma_start(vt[:], val_spill[m, :, ts(n, NT)])
                nc.vector.tensor_tensor(
                    dpredT[:, m, ts(n, NT)], pts[n][:], vt[:], ALU.subtract
                )

        # ---------------- stage 5: dpredN = transpose(dpredT) -------------
        dpredN = sb.tile([P, TK, D], FPR, tag="slotB")
        for dk in range(DK):
            for tk in range(TK):
                tp = ps.tile([P, P], FP, tag="tr", bufs=3)
                nc.tensor.transpose(
                    tp[:], dpredT[:, dk, ts(tk, P)].bitcast(FP), ident[:]
                )
                nc.vector.tensor_copy(dpredN[:, tk, ts(dk, P)], tp[:])

        # ---------------- stage 6: reload hN; g1^T partials ---------------
        hN = sb.tile([P, TK, H], FPR, tag="slotE")
        nc.sync.dma_start(
            hN[:], hN_spill[:].rearrange("tk p h -> p tk h").bitcast(FPR)
        )
        g_in = dr.tile([2, H, D], FP)
        for m in range(HK):
            for n in range(D // NT):
                pt = ps.tile([P, NT], FP, tag="mm", bufs=4)
                for k in range(TK):
                    nc.tensor.matmul(
                        pt[:], hN[:, k, ts(m, P)], dpredN[:, k, ts(n, NT)],
                        start=(k == 0), stop=(k == TK - 1),
                    )
                st = sb.tile([P, NT], FP, tag="gst", bufs=2)
                nc.vector.tensor_copy(st[:], pt[:])
                nc.sync.dma_start(g_in[1, ts(m, P), ts(n, NT)], st[:])

        # ---------------- stage 7: dh (N-layout); dpre = dh * sp ----------
        dpreN = sb.tile([P, TK, H], FPR, tag="slotE")
        for hn in range(H // NW):
            MW1N = sb.tile([P, DK, NW], FPR, tag="w2m", bufs=2)
            nc.sync.dma_start(MW1N[:], mw1n_h[hn].bitcast(FPR))
            for m in range(TK):
                pt = ps.tile([P, NW], FP, tag="mm", bufs=4)
                for k in range(DK):
                    nc.tensor.matmul(
                        pt[:], dpredT[:, k, ts(m, P)], MW1N[:, k, :],
                        start=(k == 0), stop=(k == DK - 1),
                    )
                sp_t = sb.tile([P, NW], FP, tag="sp_ring", bufs=2)
                nc.sync.dma_start(sp_t[:], sp_spill[m, :, ts(hn, NW)])
                nc.vector.tensor_tensor(
                    dpreN[:, m, ts(hn, NW)], pt[:], sp_t[:], ALU.mult
                )

        # ---------------- stage 8: keysN; g0 partials ---------------------
        keysN = sb.tile([P, TK, D], FPR, tag="slotA")
        for dk in range(DK):
            for tk in range(TK):
                tp = ps.tile([P, P], FP, tag="tr", bufs=3)
                nc.tensor.transpose(
                    tp[:], keysT[:, dk, ts(tk, P)].bitcast(FP), ident[:]
                )
                nc.vector.tensor_copy(keysN[:, tk, ts(dk, P)], tp[:])
        for m in range(HK):
            for n in range(D // NT):
                pt = ps.tile([P, NT], FP, tag="mm", bufs=4)
                for k in range(TK):
                    nc.tensor.matmul(
                        pt[:], dpreN[:, k, ts(m, P)], keysN[:, k, ts(n, NT)],
                        start=(k == 0), stop=(k == TK - 1),
                    )
                st = sb.tile([P, NT], FP, tag="gst", bufs=2)
                nc.vector.tensor_copy(st[:], pt[:])
                nc.sync.dma_start(g_in[0, ts(m, P), ts(n, NT)], st[:])

        # ---------------- stage 9: queries / retrieved / out --------------
        # Emitted BEFORE the ReduceScatter trigger so its compute overlaps
        # the collective. X2 reuses keysT's slot (free once keysN is built).
        X2 = sb.tile([P, DK, T], FPR, tag="keysT")
        nc.sync.dma_start(X2[:], xT_h[:].bitcast(FPR))
        queriesT = sb.tile([P, DK, T], FPR, tag="slotA")
        projection(X2, "qp1", "qp2", queriesT, "slotB")
        norm_inplace(queriesT)

        qhT = sb.tile([P, HK, T], FPR, tag="slotE")
        for m in range(HK):
            W0C = sb.tile([P, DK, P], FPR, tag="w4c", bufs=2)
            nc.sync.dma_start(W0C[:], mw0q_h[m].bitcast(FPR))
            for n in range(T // NT):
                pt = ps.tile([P, NT], FP, tag="mm", bufs=4)
                for k in range(DK):
                    nc.tensor.matmul(
                        pt[:], W0C[:, k, :], queriesT[:, k, ts(n, NT)],
                        start=(k == 0), stop=(k == DK - 1),
                    )
                nc.scalar.activation(qhT[:, m, ts(n, NT)], pt[:], AF.Silu)

        retrT = sb.tile([P, DK, T], FPR, tag="slotB")
        for m in range(DK):
            pts = [ps.tile([P, NT], FP, tag="mm", bufs=4, name=f"retr_ps{_n}")
                   for _n in range(T // NT)]
            for kh in range(2):
                W1C = sb.tile([P, HK // 2, P], FPR, tag="w1m", bufs=2)
                nc.sync.dma_start(W1C[:], mw1T_h[m, kh].bitcast(FPR))
                for n in range(T // NT):
                    for k8 in range(HK // 2):
                        nc.tensor.matmul(
                            pts[n][:], W1C[:, k8, :],
                            qhT[:, kh * (HK // 2) + k8, ts(n, NT)],
                            start=(kh == 0 and k8 == 0),
                            stop=(kh == 1 and k8 == HK // 2 - 1),
                        )
            for n in range(T // NT):
                nc.vector.tensor_copy(retrT[:, m, ts(n, NT)], pts[n][:])

        outN_ap = outN_h[:, :].rearrange("(tk p) d -> p tk d", p=P)
        for nd in range(D // NW):
            OWT = sb.tile([P, DK, NW], FPR, tag="w2m", bufs=2)
            nc.sync.dma_start(OWT[:], ow_h[nd].bitcast(FPR))
            for m in range(TK):
                pt = ps.tile([P, NW], FP, tag="mm", bufs=4)
                for k in range(DK):
                    nc.tensor.matmul(
                        pt[:], retrT[:, k, ts(m, P)], OWT[:, k, :],
                        start=(k == 0), stop=(k == DK - 1),
                    )
                st = sb.tile([P, NW], FP, tag="gst", bufs=2)
                nc.vector.tensor_copy(st[:], pt[:])
                nc.sync.dma_start(outN_ap[:, m, ts(nd, NW)], st[:])

        # ---------------- stage 10: ReduceScatter gradients ---------------
        g_out = dr.tile([SHARD], FP)
        nc.gpsimd.collective_compute(
            "ReduceScatter", ALU.add, replica_groups=[CORE_IDS],
            ins=[g_in.opt()], outs=[g_out.opt()],
        )
        if dbg_g_h is not None:
            nc.sync.dma_start(dbg_g_h[:], g_out[:])

        # ---------------- stage 11: gates ---------------------------------
        # z_g = x_mean @ W_g^T + b_g with x_mean as the 4-row stationary
        # operand and W_g^T streamed as the moving operand; bias added as a
        # rank-1 accumulate into the same PSUM group.
        xsum_sb = sb.tile([P, DK, B], FP, tag="xsum")
        nc.sync.dma_start(
            xsum_sb[:], xsum_out[:].rearrange("(dk p) b -> p dk b", p=P)
        )
        xsum_r = sb.tile([P, DK, B], FPR, tag="xsum_r")
        nc.vector.tensor_scalar_mul(xsum_r[:], xsum_sb[:], 1.0 / S)
        NG = D // NW
        red_parts = sb.tile([B, 3 * NG], FP, tag="redp")
        for gi, (wname, bname) in enumerate(
            (("gd", "gdb"), ("gl", "glb"), ("gm", "gmb"))
        ):
            for n in range(NG):
                GW = sb.tile([P, DK, NW], FPR, tag="w2m", bufs=2)
                nc.sync.dma_start(GW[:], gw_h[wname][n].bitcast(FPR))
                gb_t = sb.tile([1, NW], FPR, tag="gst", bufs=2)
                nc.sync.dma_start(
                    gb_t[:], gb_h[bname][:][None, ts(n, NW)].bitcast(FPR)
                )
                pz = ps.tile([B, NW], FP, tag="nrm", bufs=1)
                for k in range(DK):
                    nc.tensor.matmul(
                        pz[:], xsum_r[:, k, :], GW[:, k, :],
                        start=(k == 0), stop=False,
                    )
                nc.tensor.matmul(pz[:], ones1x4[:], gb_t[:],
                                 start=False, stop=True)
                sg_t = sb.tile([B, NW], FP, tag="gst", bufs=2)
                nc.scalar.activation(sg_t[:], pz[:], AF.Sigmoid)
                nc.vector.reduce_sum(
                    red_parts[:, gi * NG + n : gi * NG + n + 1], sg_t[:],
                    axis=mybir.AxisListType.X,
                )
        red4 = sb.tile([B, 3], FP, tag="red4")
        nc.vector.reduce_sum(
            red4[:], red_parts[:].rearrange("b (g n) -> b g n", n=NG),
            axis=mybir.AxisListType.X,
        )
        p3 = ps.tile([1, 3], FP, tag="nrm", bufs=1)
        nc.tensor.matmul(p3[:], ones_col_f[0:B, :], red4[:],
                         start=True, stop=True)
        srow = sb.tile([1, 3], FP, tag="srow")
        nc.scalar.mul(srow[:], p3[:], 1.0 / (B * D))
        pb3 = ps.tile([P, 3], FP, tag="nrm", bufs=1)
        nc.tensor.matmul(pb3[:], ones_row[:], srow[:], start=True, stop=True)
        coef = sb.tile([P, 3], FP, tag="coef")
        # coef[:,0]=1-alpha ; coef[:,1]=theta*GRAD_SCALE ; coef[:,2]=eta
        nc.vector.tensor_scalar(
            coef[:, 0:1], pb3[:, 0:1], -1.0, 1.0, ALU.mult, ALU.add
        )
        nc.vector.tensor_scalar_mul(coef[:, 1:2], pb3[:, 1:2], GRAD_SCALE)
        nc.vector.tensor_copy(coef[:, 2:3], pb3[:, 2:3])

        # ---------------- stage 12: momentum / weight update --------------
        upd_a = sb.tile([P, 2, FS], FP, tag="keysT")   # [g, mom]
        nc.sync.dma_start(upd_a[:, 0, :], g_out[:].rearrange("(p f) -> p f", p=P))
        nc.sync.dma_start(upd_a[:, 1, :], msh_h[:].rearrange("(p f) -> p f", p=P))
        upd_b = sb.tile([P, 2, FS], FP, tag="slotA")   # [w, new_m]
        nc.sync.dma_start(upd_b[:, 0, :], wsh_h[:].rearrange("(p f) -> p f", p=P))

        nc.vector.tensor_scalar_mul(upd_a[:, 0, :], upd_a[:, 0, :], coef[:, 1:2])
        nc.vector.scalar_tensor_tensor(
            upd_b[:, 1, :], upd_a[:, 1, :], coef[:, 2:3], upd_a[:, 0, :],
            ALU.mult, ALU.subtract,
        )
        nc.vector.scalar_tensor_tensor(
            upd_b[:, 0, :], upd_b[:, 0, :], coef[:, 0:1], upd_b[:, 1, :],
            ALU.mult, ALU.add,
        )
        nc.sync.dma_start(
            nmsh_h[:].rearrange("(p f) -> p f", p=P), upd_b[:, 1, :]
        )
        nc.sync.dma_start(
            nwsh_h[:].rearrange("(p f) -> p f", p=P), upd_b[:, 0, :]
        )

    nc.finalize()
    return nc


_NC_CACHE = {}


def _get_nc():
    if "nc" not in _NC_CACHE:
        _NC_CACHE["nc"] = _build_program()
    return _NC_CACHE["nc"]


def _prep_in_maps(inputs):
    f32c = lambda a: np.ascontiguousarray(np.asarray(a, dtype=np.float32))
    x = f32c(inputs["x"])

    def wcols(wT):
        # (D_in, D_out) [in,out] -> [m, p, k, c]: el = wT[k*P+p, m*P+c]
        return np.ascontiguousarray(
            wT.reshape(DK, P, DK, P).transpose(2, 1, 0, 3)
        )

    def wslices(wT, n):
        # (D_in, N_out) -> [N_out//n, p, dk, n]
        no = wT.shape[1] // n
        return np.ascontiguousarray(
            wT.reshape(DK, P, no, n).transpose(2, 1, 0, 3)
        )

    mw0T = f32c(inputs["mem_w0"].T)     # (D, H)
    mw1T = f32c(inputs["mem_w1"].T)     # (H, D)
    mw1n = f32c(inputs["mem_w1"])       # (D, H)
    shared = {
        "kp1p": wcols(f32c(inputs["kp_w1"].T)),
        "kp2p": wcols(f32c(inputs["kp_w2"].T)),
        "vp1p": wcols(f32c(inputs["vp_w1"].T)),
        "vp2p": wcols(f32c(inputs["vp_w2"].T)),
        "qp1p": wcols(f32c(inputs["qp_w1"].T)),
        "qp2p": wcols(f32c(inputs["qp_w2"].T)),
        "gdp": wslices(f32c(inputs["gd_w"].T), NW),
        "glp": wslices(f32c(inputs["gl_w"].T), NW),
        "gmp": wslices(f32c(inputs["gm_w"].T), NW),
        "gdb": f32c(inputs["gd_b"]), "glb": f32c(inputs["gl_b"]),
        "gmb": f32c(inputs["gm_b"]),
        "mw0zp": wslices(mw0T, NW),
        "mw0qp": wslices(mw0T, P),
        # (H, D) -> [m, kh, p, k8, c]: el = mw1T[(kh*8+k8)*P+p, m*P+c]
        "mw1Tp": np.ascontiguousarray(
            mw1T.reshape(2, HK // 2, P, DK, P).transpose(3, 0, 2, 1, 4)
        ),
        "mw1np": wslices(mw1n, NW),
        "owp": wslices(f32c(inputs["out_w"].T), NW),
    }
    wstack = np.concatenate(
        [f32c(inputs["mem_w0"]).reshape(H, D),
         f32c(inputs["mem_w1"]).T.reshape(H, D)]
    ).reshape(-1)
    mstack = np.concatenate(
        [f32c(inputs["mom0"]).reshape(H, D),
         f32c(inputs["mom1"]).T.reshape(H, D)]
    ).reshape(-1)

    in_maps = []
    for c in range(NCORES):
        xs = x[:, c * SB : (c + 1) * SB, :].reshape(T, D).T  # (D, T)
        m = dict(shared)
        m["xTp"] = np.ascontiguousarray(
            xs.reshape(DK, P, T).transpose(1, 0, 2)
        )
        m["wsh"] = np.ascontiguousarray(wstack[c * SHARD : (c + 1) * SHARD])
        m["msh"] = np.ascontiguousarray(mstack[c * SHARD : (c + 1) * SHARD])
        in_maps.append(m)
    return in_maps


def _unshard(results):
    out = np.empty((B, S, D), dtype=np.float32)
    for c in range(NCORES):
        o = results[c]["outN"].reshape(B, SB, D)
        out[:, c * SB : (c + 1) * SB, :] = o
    nw = np.concatenate([results[c]["nwsh"] for c in range(NCORES)]).reshape(2 * H, D)
    nm = np.concatenate([results[c]["nmsh"] for c in range(NCORES)]).reshape(2 * H, D)
    new_w0 = nw[:H]                                  # (H, D)
    new_w1 = np.ascontiguousarray(nw[H:].T)          # (D, H)
    new_m0 = nm[:H]
    new_m1 = np.ascontiguousarray(nm[H:].T)
    return out, new_w0, new_w1, new_m0, new_m1


def kernel(**inputs):
    nc = _get_nc()
    in_maps = _prep_in_maps(inputs)
    res = run_bass_kernel_spmd(nc, in_maps, CORE_IDS)
    return _unshard(res.results)


def run_traced(**inputs):
    """Like kernel() but with NTFF tracing; returns (outputs, BassKernelResults)."""
    nc = _get_nc()
    in_maps = _prep_in_maps(inputs)
    res = run_bass_kernel_spmd(nc, in_maps, CORE_IDS, trace=True)
    return _unshard(res.results), res
